# revision 17
# baseline (speedup 1.0000x reference)
"""Trainium2 Bass kernel for LocalDownsampleFlexAttn (24-head attention with
pooled-KV augmentation), head-parallel across 8 NeuronCores.

Sharding: each core owns 3 of the 24 heads. Per core:
  - QKV projections for its 3 heads (column-sliced Wq/Wk/Wv)
  - KV downsampling (4x4 spatial pooling of the 1024 image tokens -> 64)
  - attention over 1536+64 keys
  - partial output projection (row-sliced Wo); host sums the 8 partials + bo.

v2: host pre-formats x as transposed bf16 (kills the on-device PE transpose
pass and halves DMA), weights pre-cast to bf16, output written bf16.
Softmax denominators come from an all-ones [k,128] matmul so the reciprocal
is born partition-broadcast (no DRAM roundtrip). Attention + output
projection are software-pipelined: D(qg-1) matmuls interleave the scores
matmuls of qg to hide the exp() latency on the scalar engine.
"""

import numpy as np
from contextlib import ExitStack

# ---- problem constants (hardcoded per harness contract) ----
S = 1536          # sequence length
DM = 3072         # model dim
NH = 24           # total heads
HD = 128          # head dim
NCORES = 8
HPC = NH // NCORES   # heads per core = 3
CW = HPC * HD        # per-core slice width = 384
TXT = 512
IMG = 1024        # image tokens (32x32)
F = 4             # pooling factor
PK = (IMG // (F * F))  # pooled keys = 64
KALL = S + PK     # 1600 keys
NKT = DM // 128   # 24 model-dim k-tiles
NTT = S // 128    # 12 token tiles
NIT = IMG // 128  # 8 image-token tiles
NKC = (KALL + 127) // 128   # 13 key tiles (last has 64)
ASCALE = float((1.0 / HD) ** 0.5)
NQG = 3           # query groups of 512
QG = 512

_CACHE = {}


def _build_program():
    import concourse.bass as bass
    import concourse.bacc as bacc
    import concourse.tile as tile
    from concourse import mybir

    f32 = mybir.dt.float32
    bf16 = mybir.dt.bfloat16
    AF = mybir.ActivationFunctionType
    AX = mybir.AxisListType

    nc = bacc.Bacc(
        "TRN2",
        target_bir_lowering=False,
        debug=False,
        enable_asserts=False,
        num_devices=NCORES,
    )

    xT_d = nc.dram_tensor("xt", [DM, S], bf16, kind="ExternalInput").ap()
    wq_d = nc.dram_tensor("wq", [DM, CW], bf16, kind="ExternalInput").ap()
    wk_d = nc.dram_tensor("wk", [DM, CW], bf16, kind="ExternalInput").ap()
    wv_d = nc.dram_tensor("wv", [DM, CW], bf16, kind="ExternalInput").ap()
    bq_d = nc.dram_tensor("bq", [CW], f32, kind="ExternalInput").ap()
    bk_d = nc.dram_tensor("bk", [CW], f32, kind="ExternalInput").ap()
    bv_d = nc.dram_tensor("bv", [CW], bf16, kind="ExternalInput").ap()
    wo_d = nc.dram_tensor("wo", [CW, DM], bf16, kind="ExternalInput").ap()
    pmat_d = nc.dram_tensor("pmat", [IMG, PK], bf16, kind="ExternalInput").ap()
    wfull_d = nc.dram_tensor("wfull", [IMG], f32, kind="ExternalInput").ap()
    out_d = nc.dram_tensor("out", [S, DM], bf16, kind="ExternalOutput").ap()

    # engine alternation for PSUM->SBUF copies (gpsimd cannot access PSUM)
    _flip = [0]

    def copy_alt(dst, src):
        _flip[0] ^= 1
        if _flip[0]:
            nc.vector.tensor_copy(dst, src)
        else:
            nc.scalar.copy(dst, src)

    with tile.TileContext(nc) as tc, ExitStack() as ctx:
        persist = ctx.enter_context(tc.tile_pool(name="persist", bufs=1))

        # per-head per-partition biases, host-prelaid as [128, HPC]
        bq_sb = persist.tile([128, HPC], f32)
        bk_sb = persist.tile([128, HPC], f32)
        bvrow = persist.tile([1, CW], bf16)
        ones_row = persist.tile([1, 128], bf16)
        nc.vector.memset(ones_row, 1.0)
        ones_sq = persist.tile([128, 128], bf16)
        nc.vector.memset(ones_sq, 1.0)

        # persistent activations
        qT = persist.tile([128, HPC, S], bf16)          # q^T per head [d, tok]
        kT = persist.tile([128, HPC, NKC * 128], bf16)  # k_all^T per head [d, key]
        vA = persist.tile([128, HPC, NKC, HD], bf16)    # v_all per head [key, kt, d]

        pm_bf = persist.tile([128, NIT, PK], bf16)
        wfull_sb = persist.tile([128, IMG], f32)

        # ---------------- Phase B: QKV projections ----------------
        with tc.tile_pool(name="pX", bufs=1) as pX, \
             tc.tile_pool(name="pW", bufs=1) as pW:
            xT_sb = pX.tile([128, NKT, S], bf16)
            wq_sb = pW.tile([128, NKT, CW], bf16)
            wk_sb = pW.tile([128, NKT, CW], bf16)
            wv_sb = pW.tile([128, NKT, CW], bf16)

            def dma_kt_chunk(sb, dram_ap, width, kt0, nkt, eng=None):
                (eng or nc.sync).dma_start(
                    out=sb[:, kt0:kt0 + nkt, :],
                    in_=bass.AP(
                        tensor=dram_ap.tensor,
                        offset=kt0 * 128 * width,
                        ap=[[width, 128], [128 * width, nkt], [1, width]],
                    ),
                )

            # xT streams on the sync queue while all weights stream in
            # parallel on the gpsimd queue (two independent HW DMA rings)
            CH = 3
            for kt0 in range(0, NKT, CH):
                dma_kt_chunk(xT_sb, xT_d, S, kt0, CH)
            for kt0 in range(0, NKT, CH):
                dma_kt_chunk(wq_sb, wq_d, CW, kt0, CH, eng=nc.gpsimd)
                if kt0 == 3:
                    nc.gpsimd.dma_start(
                        out=bq_sb,
                        in_=bass.AP(tensor=bq_d.tensor, offset=0,
                                    ap=[[HPC, 128], [1, HPC]]),
                    )
                    nc.gpsimd.dma_start(
                        out=bk_sb,
                        in_=bass.AP(tensor=bk_d.tensor, offset=0,
                                    ap=[[HPC, 128], [1, HPC]]),
                    )
                    nc.gpsimd.dma_start(out=bvrow, in_=bv_d[None, :])
            for kt0 in range(0, NKT, 6):
                dma_kt_chunk(wk_sb, wk_d, CW, kt0, 6, eng=nc.gpsimd)
            for kt0 in range(0, NKT, 6):
                dma_kt_chunk(wv_sb, wv_d, CW, kt0, 6, eng=nc.gpsimd)
            nc.gpsimd.dma_start(
                out=pm_bf,
                in_=bass.AP(tensor=pmat_d.tensor, offset=0,
                            ap=[[PK, 128], [128 * PK, NIT], [1, PK]]),
            )
            nc.gpsimd.dma_start(
                out=wfull_sb,
                in_=bass.AP(tensor=wfull_d.tensor, offset=0, ap=[[0, 128], [1, IMG]]),
            )

            with tc.tile_pool(name="pBps", bufs=2, space="PSUM") as pBps, \
                 tc.tile_pool(name="pVps", bufs=2, space="PSUM") as pVps:
                # q pass for h0/h1 interleaved kt-outer (paces with the DMA stream)
                ps0 = pBps.tile([128, 1536], f32, tag="qk")
                ps1 = pBps.tile([128, 1536], f32, tag="qk")
                for kt in range(NKT):
                    for h, ps in ((0, ps0), (1, ps1)):
                        for c in range(3):
                            nc.tensor.matmul(
                                ps[:, c * 512:(c + 1) * 512],
                                wq_sb[:, kt, h * 128:(h + 1) * 128],
                                xT_sb[:, kt, c * 512:(c + 1) * 512],
                                start=(kt == 0),
                                stop=(kt == NKT - 1),
                            )
                for h, ps in ((0, ps0), (1, ps1)):
                    for c in range(3):
                        nc.scalar.activation(
                            qT[:, h, c * 512:(c + 1) * 512],
                            ps[:, c * 512:(c + 1) * 512],
                            AF.Identity,
                            bias=bq_sb[:, h:h + 1],
                            scale=1.0,
                        )

                # q h2, then k h0..2 (weights resident by now)
                for w_sb, b_sb, dst, hs in (
                    (wq_sb, bq_sb, qT, (2,)),
                    (wk_sb, bk_sb, kT, (0, 1, 2)),
                ):
                    for h in hs:
                        ps = pBps.tile([128, 1536], f32, tag="qk")
                        for kt in range(NKT):
                            for c in range(3):
                                nc.tensor.matmul(
                                    ps[:, c * 512:(c + 1) * 512],
                                    w_sb[:, kt, h * 128:(h + 1) * 128],
                                    xT_sb[:, kt, c * 512:(c + 1) * 512],
                                    start=(kt == 0),
                                    stop=(kt == NKT - 1),
                                )
                        for c in range(3):
                            nc.scalar.activation(
                                dst[:, h, c * 512:(c + 1) * 512],
                                ps[:, c * 512:(c + 1) * 512],
                                AF.Identity,
                                bias=b_sb[:, h:h + 1],
                                scale=1.0,
                            )

                # v (natural layout), bias via K=1 outer product
                for tt in range(NTT):
                    psv = pVps.tile([128, CW], f32, tag="v")
                    nc.tensor.matmul(psv, ones_row, bvrow, start=True, stop=False)
                    for kt in range(NKT):
                        nc.tensor.matmul(
                            psv,
                            xT_sb[:, kt, tt * 128:(tt + 1) * 128],
                            wv_sb[:, kt, :],
                            start=False,
                            stop=(kt == NKT - 1),
                        )
                    for h in range(HPC):
                        copy_alt(vA[:, h, tt, :], psv[:, h * 128:(h + 1) * 128])

                # pooled v rows (keys 1536:1600 -> tile 12, rows 0:64)
                for h in range(HPC):
                    psp = pVps.tile([128, CW], f32, tag="v")
                    for it in range(NIT):
                        nc.tensor.matmul(
                            psp[:PK, :HD],
                            pm_bf[:, it, :],
                            vA[:, h, (TXT // 128) + it, :],
                            start=(it == 0),
                            stop=(it == NIT - 1),
                        )
                    copy_alt(vA[:PK, h, NKC - 1, :], psp[:PK, :HD])

                # pooled k columns (kT[:, h, 1536:1600]) via DVE weighted reduce
                with tc.tile_pool(name="pKp", bufs=2) as pKp:
                    for h in range(HPC):
                        tmpw = pKp.tile([128, IMG], f32, tag="tmpw")
                        for R in range(8):
                            nc.vector.tensor_mul(
                                tmpw[:, R * 128:(R + 1) * 128].rearrange(
                                    "p (C i j) -> p C i j", C=8, i=4),
                                kT[:, h, TXT + R * 128:TXT + (R + 1) * 128].rearrange(
                                    "p (i C j) -> p C i j", i=4, C=8),
                                wfull_sb[:, R * 128:(R + 1) * 128].rearrange(
                                    "p (i C j) -> p C i j", i=4, C=8),
                            )
                        pooled = pKp.tile([128, PK], f32, tag="pooled")
                        nc.vector.reduce_sum(
                            pooled,
                            tmpw.rearrange("p (rc ij) -> p rc ij", ij=F * F),
                            axis=AX.X,
                        )
                        copy_alt(kT[:, h, S:S + PK], pooled)

        # ---------------- Phase C+D: attention + output projection ----------------
        # scores computed transposed ([key, query]); softmax sums via an
        # all-ones [k,128] matmul so the denominator lands broadcast across
        # all partitions (reciprocal + multiply stay on-chip, no DRAM bounce).
        # The output projection for query group qg-1 is interleaved between
        # the scores matmuls of qg to keep the PE busy while exp() runs.
        with tc.tile_pool(name="pWo", bufs=1) as pWo, \
             tc.tile_pool(name="pP", bufs=4) as pP, \
             tc.tile_pool(name="pAcc", bufs=3) as pAcc, \
             tc.tile_pool(name="pN", bufs=2) as pN, \
             tc.tile_pool(name="pA", bufs=3) as pA, \
             tc.tile_pool(name="pO", bufs=2) as pO, \
             tc.tile_pool(name="pSC", bufs=2, space="PSUM") as pSC, \
             tc.tile_pool(name="pRow", bufs=2, space="PSUM") as pRow, \
             tc.tile_pool(name="pPV", bufs=2, space="PSUM") as pPV, \
             tc.tile_pool(name="pSO", bufs=2, space="PSUM") as pSO:
            wo_sb = pWo.tile([128, HPC, DM], bf16)
            nc.sync.dma_start(
                out=wo_sb,
                in_=bass.AP(tensor=wo_d.tensor, offset=0,
                            ap=[[DM, 128], [128 * DM, HPC], [1, DM]]),
            )

            def d_groups_for(qg, attn_sb):
                """Output-projection work for query group qg as a list of
                closures (one per (qt, g) psum group: 3 matmuls + copy/DMA)."""
                groups = []
                outsb = {}
                for qi in range(4):
                    qt = qg * 4 + qi
                    osb = pO.tile([128, DM], bf16, tag="osb")
                    outsb[qt] = osb
                    for g in range(6):
                        def work(qt=qt, qi=qi, g=g, osb=osb):
                            pso = pSO.tile([128, 512], f32, tag="o")
                            for h in range(HPC):
                                nc.tensor.matmul(
                                    pso,
                                    attn_sb[:, h, qi * 128:(qi + 1) * 128],
                                    wo_sb[:, h, g * 512:(g + 1) * 512],
                                    start=(h == 0),
                                    stop=(h == HPC - 1),
                                )
                            copy_alt(osb[:, g * 512:(g + 1) * 512], pso)
                            if g == 5:
                                nc.sync.dma_start(
                                    out=out_d[qt * 128:(qt + 1) * 128, :],
                                    in_=osb,
                                )
                        groups.append(work)
                return groups

            pending = []   # leftover D work from the previous query group
            for qg in range(NQG):
                qsl = slice(qg * QG, (qg + 1) * QG)
                attn_sb = pA.tile([128, HPC, QG], bf16, tag="attn")
                # interleave pending D groups among the 39 scores matmuls
                nslots = HPC * NKC
                emitted = [0]

                def drip(slot):
                    want = (slot + 1) * len(pending) // nslots
                    while emitted[0] < want:
                        pending[emitted[0]]()
                        emitted[0] += 1

                for h in range(HPC):
                    probsT = pP.tile([128, NKC, QG], bf16, tag="probsT")
                    for c in range(NKC):
                        cs = 128 if c < NKC - 1 else PK
                        psc = pSC.tile([128, QG], f32, tag="sc")
                        nc.tensor.matmul(
                            psc[:cs, :],
                            kT[:, h, c * 128:c * 128 + cs],
                            qT[:, h, qsl],
                            start=True,
                            stop=True,
                        )
                        drip(h * NKC + c)
                        nc.scalar.activation(
                            probsT[:cs, c, :],
                            psc[:cs, :],
                            AF.Exp,
                            bias=0.0,
                            scale=ASCALE,
                        )
                    # softmax denominators, broadcast across partitions
                    prow = pRow.tile([128, QG], f32, tag="srow")
                    for c in range(NKC):
                        cs = 128 if c < NKC - 1 else PK
                        nc.tensor.matmul(
                            prow,
                            ones_sq[:cs, :],
                            probsT[:cs, c, :],
                            start=(c == 0),
                            stop=(c == NKC - 1),
                        )
                    rsb = pN.tile([128, QG], f32, tag="rsb")
                    nc.vector.reciprocal_approx_fast(rsb, prow)
                    # pv
                    ppv = pPV.tile([128, QG], f32, tag="pv")
                    for c in range(NKC):
                        cs = 128 if c < NKC - 1 else PK
                        nc.tensor.matmul(
                            ppv,
                            vA[:cs, h, c, :],
                            probsT[:cs, c, :],
                            start=(c == 0),
                            stop=(c == NKC - 1),
                        )
                    nc.vector.tensor_mul(attn_sb[:, h, :], ppv, rsb)
                # flush any stragglers, queue this group's D work
                for w in pending[emitted[0]:]:
                    w()
                pending = d_groups_for(qg, attn_sb)
            for w in pending:
                w()

    nc.compile()
    return nc


def _get_program():
    if "nc" not in _CACHE:
        _CACHE["nc"] = _build_program()
    return _CACHE["nc"]


def _prep_in_maps(hidden_states, Wq, bq, Wk, bk, Wv, bv, Wo, spatial_weight):
    import ml_dtypes
    bf = ml_dtypes.bfloat16

    x = np.asarray(hidden_states, dtype=np.float32).reshape(S, DM)
    xT = np.ascontiguousarray(x.T).astype(bf)          # [DM, S] bf16
    WqB = np.asarray(Wq, dtype=np.float32).astype(bf)
    WkB = np.asarray(Wk, dtype=np.float32).astype(bf)
    WvB = np.asarray(Wv, dtype=np.float32).astype(bf)
    WoB = np.asarray(Wo, dtype=np.float32).astype(bf)
    bq = np.asarray(bq, dtype=np.float32)
    bk = np.asarray(bk, dtype=np.float32)
    bvB = np.asarray(bv, dtype=np.float32).astype(bf)

    w = np.asarray(spatial_weight, dtype=np.float32).reshape(F, F)  # [i, j]
    # wfull[t] for t = 128R + 32i + 4C + j  -> broadcast w over (R, C)
    wfull = np.ascontiguousarray(
        np.broadcast_to(w[None, :, None, :], (8, F, 8, F)).reshape(IMG)
    )
    # pmat[t, R*8+C] = w[i, j] for t in block (R, C)
    pmat = np.zeros((8, F, 8, F, 8, 8), dtype=np.float32)
    for R in range(8):
        for C in range(8):
            pmat[R, :, C, :, R, C] = w
    pmat = np.ascontiguousarray(pmat.reshape(IMG, PK)).astype(bf)

    in_maps = []
    for c in range(NCORES):
        sl = slice(c * CW, (c + 1) * CW)
        in_maps.append({
            "xt": xT,
            "wq": np.ascontiguousarray(WqB[:, sl]),
            "wk": np.ascontiguousarray(WkB[:, sl]),
            "wv": np.ascontiguousarray(WvB[:, sl]),
            "bq": np.ascontiguousarray(bq[sl].reshape(HPC, 128).T),
            "bk": np.ascontiguousarray(bk[sl].reshape(HPC, 128).T),
            "bv": np.ascontiguousarray(bvB[sl]),
            "wo": np.ascontiguousarray(WoB[sl, :]),
            "pmat": pmat,
            "wfull": wfull,
        })
    return in_maps


def _run(inputs, trace=False, trace_kwargs=None):
    from concourse import bass_utils

    nc = _get_program()
    in_maps = _prep_in_maps(
        inputs["hidden_states"], inputs["Wq"], inputs["bq"], inputs["Wk"],
        inputs["bk"], inputs["Wv"], inputs["bv"], inputs["Wo"],
        inputs["spatial_weight"],
    )
    res = bass_utils.run_bass_kernel_spmd(
        nc, in_maps, list(range(NCORES)), trace=trace,
        **(trace_kwargs or {}),
    )
    partial = np.zeros((S, DM), dtype=np.float32)
    for r in res.results:
        partial += np.asarray(r["out"]).astype(np.float32)
    out = partial + np.asarray(inputs["bo"], dtype=np.float32)[None, :]
    return out.reshape(1, S, DM).astype(np.float32), res


def kernel(**inputs):
    h = int(inputs.get("height", 32))
    w = int(inputs.get("width", 32))
    assert h == 32 and w == 32, (h, w)
    out, _ = _run(inputs, trace=False)
    return out


# revision 18
# speedup vs baseline: 1.0571x; 1.0571x over previous
"""Trainium2 Bass kernel for LocalDownsampleFlexAttn (24-head attention with
pooled-KV augmentation), head-parallel across 8 NeuronCores.

Sharding: each core owns 3 of the 24 heads. Per core:
  - QKV projections for its 3 heads (column-sliced Wq/Wk/Wv)
  - KV downsampling (4x4 spatial pooling of the 1024 image tokens -> 64)
  - attention over 1536+64 keys
  - partial output projection (row-sliced Wo); host sums the 8 partials + bo.

v2: host pre-formats x as transposed bf16 (kills the on-device PE transpose
pass and halves DMA), weights pre-cast to bf16, output written bf16.
Softmax denominators come from an all-ones [k,128] matmul so the reciprocal
is born partition-broadcast (no DRAM roundtrip). Attention + output
projection are software-pipelined: D(qg-1) matmuls interleave the scores
matmuls of qg to hide the exp() latency on the scalar engine.
"""

import numpy as np
from contextlib import ExitStack

# ---- problem constants (hardcoded per harness contract) ----
S = 1536          # sequence length
DM = 3072         # model dim
NH = 24           # total heads
HD = 128          # head dim
NCORES = 8
HPC = NH // NCORES   # heads per core = 3
CW = HPC * HD        # per-core slice width = 384
TXT = 512
IMG = 1024        # image tokens (32x32)
F = 4             # pooling factor
PK = (IMG // (F * F))  # pooled keys = 64
KALL = S + PK     # 1600 keys
NKT = DM // 128   # 24 model-dim k-tiles
NTT = S // 128    # 12 token tiles
NIT = IMG // 128  # 8 image-token tiles
NKC = (KALL + 127) // 128   # 13 key tiles (last has 64)
ASCALE = float((1.0 / HD) ** 0.5)
NQG = 3           # query groups of 512
QG = 512

_CACHE = {}


def _build_program():
    import concourse.bass as bass
    import concourse.bacc as bacc
    import concourse.tile as tile
    from concourse import mybir

    f32 = mybir.dt.float32
    bf16 = mybir.dt.bfloat16
    AF = mybir.ActivationFunctionType
    AX = mybir.AxisListType

    nc = bacc.Bacc(
        "TRN2",
        target_bir_lowering=False,
        debug=False,
        enable_asserts=False,
        num_devices=NCORES,
    )

    xT_d = nc.dram_tensor("xt", [DM, S], bf16, kind="ExternalInput").ap()
    wq_d = nc.dram_tensor("wq", [DM, CW], bf16, kind="ExternalInput").ap()
    wk_d = nc.dram_tensor("wk", [DM, CW], bf16, kind="ExternalInput").ap()
    wv_d = nc.dram_tensor("wv", [DM, CW], bf16, kind="ExternalInput").ap()
    bq_d = nc.dram_tensor("bq", [CW], f32, kind="ExternalInput").ap()
    bk_d = nc.dram_tensor("bk", [CW], f32, kind="ExternalInput").ap()
    bv_d = nc.dram_tensor("bv", [CW], bf16, kind="ExternalInput").ap()
    wo_d = nc.dram_tensor("wo", [CW, DM], bf16, kind="ExternalInput").ap()
    pmat_d = nc.dram_tensor("pmat", [IMG, PK], bf16, kind="ExternalInput").ap()
    wfull_d = nc.dram_tensor("wfull", [IMG], f32, kind="ExternalInput").ap()
    out_d = nc.dram_tensor("out", [S, DM], bf16, kind="ExternalOutput").ap()

    # engine alternation for PSUM->SBUF copies (gpsimd cannot access PSUM)
    _flip = [0]

    def copy_alt(dst, src):
        _flip[0] ^= 1
        if _flip[0]:
            nc.vector.tensor_copy(dst, src)
        else:
            nc.scalar.copy(dst, src)

    with tile.TileContext(nc) as tc, ExitStack() as ctx:
        persist = ctx.enter_context(tc.tile_pool(name="persist", bufs=1))

        # per-head per-partition biases, host-prelaid as [128, HPC]
        bq_sb = persist.tile([128, HPC], f32)
        bk_sb = persist.tile([128, HPC], f32)
        bvrow = persist.tile([1, CW], bf16)
        ones_row = persist.tile([1, 128], bf16)
        nc.vector.memset(ones_row, 1.0)
        ones_sq = persist.tile([128, 128], bf16)
        nc.vector.memset(ones_sq, 1.0)

        # persistent activations
        qT = persist.tile([128, HPC, S], bf16)          # q^T per head [d, tok]
        kT = persist.tile([128, HPC, NKC * 128], bf16)  # k_all^T per head [d, key]
        vA = persist.tile([128, HPC, NKC, HD], bf16)    # v_all per head [key, kt, d]

        pm_bf = persist.tile([128, NIT, PK], bf16)
        wfull_sb = persist.tile([128, IMG], f32)

        # ---------------- Phase B: QKV projections ----------------
        with tc.tile_pool(name="pX", bufs=1) as pX, \
             tc.tile_pool(name="pW", bufs=1) as pW:
            xT_sb = pX.tile([128, NKT, S], bf16)
            wq_sb = pW.tile([128, NKT, CW], bf16)
            wk_sb = pW.tile([128, NKT, CW], bf16)
            wv_sb = pW.tile([128, NKT, CW], bf16)

            def dma_kt_chunk(sb, dram_ap, width, kt0, nkt, eng=None):
                (eng or nc.sync).dma_start(
                    out=sb[:, kt0:kt0 + nkt, :],
                    in_=bass.AP(
                        tensor=dram_ap.tensor,
                        offset=kt0 * 128 * width,
                        ap=[[width, 128], [128 * width, nkt], [1, width]],
                    ),
                )

            # interleave wq + xT chunks so the q pass can start immediately
            # (one queue: aggregate HBM read BW is ~360GB/s regardless);
            # wk/wv/pm/wfull follow (needed later)
            CH = 2
            for kt0 in range(0, NKT, CH):
                dma_kt_chunk(wq_sb, wq_d, CW, kt0, CH)
                dma_kt_chunk(xT_sb, xT_d, S, kt0, CH)
                if kt0 == 4:
                    nc.gpsimd.dma_start(
                        out=bq_sb,
                        in_=bass.AP(tensor=bq_d.tensor, offset=0,
                                    ap=[[HPC, 128], [1, HPC]]),
                    )
                    nc.gpsimd.dma_start(
                        out=bk_sb,
                        in_=bass.AP(tensor=bk_d.tensor, offset=0,
                                    ap=[[HPC, 128], [1, HPC]]),
                    )
                    nc.gpsimd.dma_start(out=bvrow, in_=bv_d[None, :])
            for kt0 in range(0, NKT, 6):
                dma_kt_chunk(wk_sb, wk_d, CW, kt0, 6)
            for kt0 in range(0, NKT, 6):
                dma_kt_chunk(wv_sb, wv_d, CW, kt0, 6)
            nc.gpsimd.dma_start(
                out=pm_bf,
                in_=bass.AP(tensor=pmat_d.tensor, offset=0,
                            ap=[[PK, 128], [128 * PK, NIT], [1, PK]]),
            )
            nc.gpsimd.dma_start(
                out=wfull_sb,
                in_=bass.AP(tensor=wfull_d.tensor, offset=0, ap=[[0, 128], [1, IMG]]),
            )

            with tc.tile_pool(name="pBps", bufs=2, space="PSUM") as pBps, \
                 tc.tile_pool(name="pVps", bufs=2, space="PSUM") as pVps:
                # q pass for h0/h1 interleaved kt-outer (paces with the DMA stream)
                ps0 = pBps.tile([128, 1536], f32, tag="qk")
                ps1 = pBps.tile([128, 1536], f32, tag="qk")
                for kt in range(NKT):
                    for h, ps in ((0, ps0), (1, ps1)):
                        for c in range(3):
                            nc.tensor.matmul(
                                ps[:, c * 512:(c + 1) * 512],
                                wq_sb[:, kt, h * 128:(h + 1) * 128],
                                xT_sb[:, kt, c * 512:(c + 1) * 512],
                                start=(kt == 0),
                                stop=(kt == NKT - 1),
                            )
                for h, ps in ((0, ps0), (1, ps1)):
                    for c in range(3):
                        nc.scalar.activation(
                            qT[:, h, c * 512:(c + 1) * 512],
                            ps[:, c * 512:(c + 1) * 512],
                            AF.Identity,
                            bias=bq_sb[:, h:h + 1],
                            scale=1.0,
                        )

                # q h2, then k h0..2 (weights resident by now)
                for w_sb, b_sb, dst, hs in (
                    (wq_sb, bq_sb, qT, (2,)),
                    (wk_sb, bk_sb, kT, (0, 1, 2)),
                ):
                    for h in hs:
                        ps = pBps.tile([128, 1536], f32, tag="qk")
                        for kt in range(NKT):
                            for c in range(3):
                                nc.tensor.matmul(
                                    ps[:, c * 512:(c + 1) * 512],
                                    w_sb[:, kt, h * 128:(h + 1) * 128],
                                    xT_sb[:, kt, c * 512:(c + 1) * 512],
                                    start=(kt == 0),
                                    stop=(kt == NKT - 1),
                                )
                        for c in range(3):
                            nc.scalar.activation(
                                dst[:, h, c * 512:(c + 1) * 512],
                                ps[:, c * 512:(c + 1) * 512],
                                AF.Identity,
                                bias=b_sb[:, h:h + 1],
                                scale=1.0,
                            )

                # v (natural layout), bias via K=1 outer product
                for tt in range(NTT):
                    psv = pVps.tile([128, CW], f32, tag="v")
                    nc.tensor.matmul(psv, ones_row, bvrow, start=True, stop=False)
                    for kt in range(NKT):
                        nc.tensor.matmul(
                            psv,
                            xT_sb[:, kt, tt * 128:(tt + 1) * 128],
                            wv_sb[:, kt, :],
                            start=False,
                            stop=(kt == NKT - 1),
                        )
                    for h in range(HPC):
                        copy_alt(vA[:, h, tt, :], psv[:, h * 128:(h + 1) * 128])

                # pooled v rows (keys 1536:1600 -> tile 12, rows 0:64)
                for h in range(HPC):
                    psp = pVps.tile([128, CW], f32, tag="v")
                    for it in range(NIT):
                        nc.tensor.matmul(
                            psp[:PK, :HD],
                            pm_bf[:, it, :],
                            vA[:, h, (TXT // 128) + it, :],
                            start=(it == 0),
                            stop=(it == NIT - 1),
                        )
                    copy_alt(vA[:PK, h, NKC - 1, :], psp[:PK, :HD])

                # pooled k columns (kT[:, h, 1536:1600]) via DVE weighted reduce
                with tc.tile_pool(name="pKp", bufs=2) as pKp:
                    for h in range(HPC):
                        tmpw = pKp.tile([128, IMG], f32, tag="tmpw")
                        for R in range(8):
                            nc.vector.tensor_mul(
                                tmpw[:, R * 128:(R + 1) * 128].rearrange(
                                    "p (C i j) -> p C i j", C=8, i=4),
                                kT[:, h, TXT + R * 128:TXT + (R + 1) * 128].rearrange(
                                    "p (i C j) -> p C i j", i=4, C=8),
                                wfull_sb[:, R * 128:(R + 1) * 128].rearrange(
                                    "p (i C j) -> p C i j", i=4, C=8),
                            )
                        pooled = pKp.tile([128, PK], f32, tag="pooled")
                        nc.vector.reduce_sum(
                            pooled,
                            tmpw.rearrange("p (rc ij) -> p rc ij", ij=F * F),
                            axis=AX.X,
                        )
                        copy_alt(kT[:, h, S:S + PK], pooled)

        # ---------------- Phase C+D: attention + output projection ----------------
        # scores computed transposed ([key, query]); softmax sums via an
        # all-ones [k,128] matmul so the denominator lands broadcast across
        # all partitions (reciprocal + multiply stay on-chip, no DRAM bounce).
        # The output projection for query group qg-1 is interleaved between
        # the scores matmuls of qg to keep the PE busy while exp() runs.
        with tc.tile_pool(name="pWo", bufs=1) as pWo, \
             tc.tile_pool(name="pP", bufs=4) as pP, \
             tc.tile_pool(name="pAcc", bufs=3) as pAcc, \
             tc.tile_pool(name="pN", bufs=2) as pN, \
             tc.tile_pool(name="pA", bufs=3) as pA, \
             tc.tile_pool(name="pO", bufs=2) as pO, \
             tc.tile_pool(name="pSC", bufs=2, space="PSUM") as pSC, \
             tc.tile_pool(name="pRow", bufs=2, space="PSUM") as pRow, \
             tc.tile_pool(name="pPV", bufs=2, space="PSUM") as pPV, \
             tc.tile_pool(name="pSO", bufs=2, space="PSUM") as pSO:
            wo_sb = pWo.tile([128, HPC, DM], bf16)
            nc.sync.dma_start(
                out=wo_sb,
                in_=bass.AP(tensor=wo_d.tensor, offset=0,
                            ap=[[DM, 128], [128 * DM, HPC], [1, DM]]),
            )

            def d_groups_for(qg, attn_sb):
                """Output-projection work for query group qg as a list of
                closures (one per (qt, g) psum group: 3 matmuls + copy/DMA)."""
                groups = []
                outsb = {}
                for qi in range(4):
                    qt = qg * 4 + qi
                    osb = pO.tile([128, DM], bf16, tag="osb")
                    outsb[qt] = osb
                    for g in range(6):
                        def work(qt=qt, qi=qi, g=g, osb=osb):
                            pso = pSO.tile([128, 512], f32, tag="o")
                            for h in range(HPC):
                                nc.tensor.matmul(
                                    pso,
                                    attn_sb[:, h, qi * 128:(qi + 1) * 128],
                                    wo_sb[:, h, g * 512:(g + 1) * 512],
                                    start=(h == 0),
                                    stop=(h == HPC - 1),
                                )
                            copy_alt(osb[:, g * 512:(g + 1) * 512], pso)
                            if g == 5:
                                nc.sync.dma_start(
                                    out=out_d[qt * 128:(qt + 1) * 128, :],
                                    in_=osb,
                                )
                        groups.append(work)
                return groups

            pending = []   # leftover D work from the previous query group
            for qg in range(NQG):
                qsl = slice(qg * QG, (qg + 1) * QG)
                attn_sb = pA.tile([128, HPC, QG], bf16, tag="attn")
                # interleave pending D groups among the 39 scores matmuls
                nslots = HPC * NKC
                emitted = [0]

                def drip(slot):
                    want = (slot + 1) * len(pending) // nslots
                    while emitted[0] < want:
                        pending[emitted[0]]()
                        emitted[0] += 1

                for h in range(HPC):
                    probsT = pP.tile([128, NKC, QG], bf16, tag="probsT")
                    for c in range(NKC):
                        cs = 128 if c < NKC - 1 else PK
                        psc = pSC.tile([128, QG], f32, tag="sc")
                        nc.tensor.matmul(
                            psc[:cs, :],
                            kT[:, h, c * 128:c * 128 + cs],
                            qT[:, h, qsl],
                            start=True,
                            stop=True,
                        )
                        drip(h * NKC + c)
                        nc.scalar.activation(
                            probsT[:cs, c, :],
                            psc[:cs, :],
                            AF.Exp,
                            bias=0.0,
                            scale=ASCALE,
                        )
                    # softmax denominators, broadcast across partitions
                    prow = pRow.tile([128, QG], f32, tag="srow")
                    for c in range(NKC):
                        cs = 128 if c < NKC - 1 else PK
                        nc.tensor.matmul(
                            prow,
                            ones_sq[:cs, :],
                            probsT[:cs, c, :],
                            start=(c == 0),
                            stop=(c == NKC - 1),
                        )
                    rsb = pN.tile([128, QG], f32, tag="rsb")
                    nc.vector.reciprocal_approx_fast(rsb, prow)
                    # pv
                    ppv = pPV.tile([128, QG], f32, tag="pv")
                    for c in range(NKC):
                        cs = 128 if c < NKC - 1 else PK
                        nc.tensor.matmul(
                            ppv,
                            vA[:cs, h, c, :],
                            probsT[:cs, c, :],
                            start=(c == 0),
                            stop=(c == NKC - 1),
                        )
                    nc.vector.tensor_mul(attn_sb[:, h, :], ppv, rsb)
                # flush any stragglers, queue this group's D work
                for w in pending[emitted[0]:]:
                    w()
                pending = d_groups_for(qg, attn_sb)
            for w in pending:
                w()

    nc.compile()
    return nc


def _get_program():
    if "nc" not in _CACHE:
        _CACHE["nc"] = _build_program()
    return _CACHE["nc"]


def _prep_in_maps(hidden_states, Wq, bq, Wk, bk, Wv, bv, Wo, spatial_weight):
    import ml_dtypes
    bf = ml_dtypes.bfloat16

    x = np.asarray(hidden_states, dtype=np.float32).reshape(S, DM)
    xT = np.ascontiguousarray(x.T).astype(bf)          # [DM, S] bf16
    WqB = np.asarray(Wq, dtype=np.float32).astype(bf)
    WkB = np.asarray(Wk, dtype=np.float32).astype(bf)
    WvB = np.asarray(Wv, dtype=np.float32).astype(bf)
    WoB = np.asarray(Wo, dtype=np.float32).astype(bf)
    bq = np.asarray(bq, dtype=np.float32)
    bk = np.asarray(bk, dtype=np.float32)
    bvB = np.asarray(bv, dtype=np.float32).astype(bf)

    w = np.asarray(spatial_weight, dtype=np.float32).reshape(F, F)  # [i, j]
    # wfull[t] for t = 128R + 32i + 4C + j  -> broadcast w over (R, C)
    wfull = np.ascontiguousarray(
        np.broadcast_to(w[None, :, None, :], (8, F, 8, F)).reshape(IMG)
    )
    # pmat[t, R*8+C] = w[i, j] for t in block (R, C)
    pmat = np.zeros((8, F, 8, F, 8, 8), dtype=np.float32)
    for R in range(8):
        for C in range(8):
            pmat[R, :, C, :, R, C] = w
    pmat = np.ascontiguousarray(pmat.reshape(IMG, PK)).astype(bf)

    in_maps = []
    for c in range(NCORES):
        sl = slice(c * CW, (c + 1) * CW)
        in_maps.append({
            "xt": xT,
            "wq": np.ascontiguousarray(WqB[:, sl]),
            "wk": np.ascontiguousarray(WkB[:, sl]),
            "wv": np.ascontiguousarray(WvB[:, sl]),
            "bq": np.ascontiguousarray(bq[sl].reshape(HPC, 128).T),
            "bk": np.ascontiguousarray(bk[sl].reshape(HPC, 128).T),
            "bv": np.ascontiguousarray(bvB[sl]),
            "wo": np.ascontiguousarray(WoB[sl, :]),
            "pmat": pmat,
            "wfull": wfull,
        })
    return in_maps


def _run(inputs, trace=False, trace_kwargs=None):
    from concourse import bass_utils

    nc = _get_program()
    in_maps = _prep_in_maps(
        inputs["hidden_states"], inputs["Wq"], inputs["bq"], inputs["Wk"],
        inputs["bk"], inputs["Wv"], inputs["bv"], inputs["Wo"],
        inputs["spatial_weight"],
    )
    res = bass_utils.run_bass_kernel_spmd(
        nc, in_maps, list(range(NCORES)), trace=trace,
        **(trace_kwargs or {}),
    )
    partial = np.zeros((S, DM), dtype=np.float32)
    for r in res.results:
        partial += np.asarray(r["out"]).astype(np.float32)
    out = partial + np.asarray(inputs["bo"], dtype=np.float32)[None, :]
    return out.reshape(1, S, DM).astype(np.float32), res


def kernel(**inputs):
    h = int(inputs.get("height", 32))
    w = int(inputs.get("width", 32))
    assert h == 32 and w == 32, (h, w)
    out, _ = _run(inputs, trace=False)
    return out
